# revision 3
# baseline (speedup 1.0000x reference)
import numpy as np
import jax
import jax.numpy as jnp

B = 8192        # graphs
NPG = 39        # nodes per graph
N = B * NPG
NC = 8          # neuron cores
GPC = B // NC   # graphs per core
NEG = 0.2

_DIAG = np.arange(NPG)
_BF = jnp.bfloat16


def _gat(x, C, W, a_s, a_d, b):
    # x [G,39,fi]; C [G,39,39] with C[g,d,s] = edge count s->d (incl self loop)
    G = x.shape[0]
    fi = x.shape[2]
    fo = W.shape[1]
    Wa = jnp.concatenate([W, (W @ a_s)[:, None], (W @ a_d)[:, None]], axis=1)
    H = x.reshape(G * NPG, fi) @ Wa                  # [G*39, fo+2]
    h = H[:, :fo].reshape(G, NPG, fo)
    s = H[:, fo].reshape(G, NPG)
    d = H[:, fo + 1].reshape(G, NPG)
    # any per-graph upper bound m works for softmax stability (cancels in
    # the normalization); relu(max s + max d) >= max lrelu(s+d) and avoids
    # the masked [G,39,39] max pass.
    m = jax.nn.relu(jnp.max(s, 1) + jnp.max(d, 1))[:, None, None]
    E = jax.nn.leaky_relu(s[:, None, :] + d[:, :, None], NEG)   # [G,d,s]
    w = (C * jnp.exp(E - m)).astype(_BF)             # unnormalized weights
    # contract once against [h | 1]: yields both sum(w*h) and denom=sum(w)
    hp = jnp.concatenate(
        [h, jnp.ones((G, NPG, 1), jnp.float32)], axis=2).astype(_BF)
    raw = jax.lax.dot_general(
        w, hp, (((2,), (1,)), ((0,), (0,))),
        preferred_element_type=jnp.float32)          # [G,d,fo+1]
    out = raw[:, :, :fo] / (raw[:, :, fo:] + 1e-16)
    return jax.nn.relu(out + b)


def _fwd(x, C, params):
    (W1, as1, ad1, b1, W2, as2, ad2, b2, W3, as3, ad3, b3,
     W4, as4, ad4, b4, lw1, lb1, lw2, lb2, lw3, lb3) = params
    G = x.shape[0]
    h1 = _gat(x[..., None], C, W1, as1, ad1, b1)
    h2 = _gat(h1, C, W2, as2, ad2, b2)
    h3 = _gat(h2, C, W3, as3, ad3, b3)
    h4 = _gat(h3, C, W4, as4, ad4, b4)
    f = jnp.concatenate([
        x.astype(_BF), h1.reshape(G, -1).astype(_BF),
        h2.reshape(G, -1).astype(_BF), h3.reshape(G, -1).astype(_BF),
        h4.reshape(G, -1).astype(_BF),
        jnp.max(x, axis=1, keepdims=True).astype(_BF),
        jnp.max(h1, axis=1).astype(_BF), jnp.max(h2, axis=1).astype(_BF),
        jnp.max(h3, axis=1).astype(_BF),
        jnp.max(h4, axis=1).astype(_BF)], axis=1)    # [G,4560] bf16
    f = jax.nn.relu(jnp.dot(f, lw1.astype(_BF),
                            preferred_element_type=jnp.float32) + lb1)
    f = jax.nn.relu(jnp.dot(f.astype(_BF), lw2.astype(_BF),
                            preferred_element_type=jnp.float32) + lb2)
    return jnp.dot(f.astype(_BF), lw3.astype(_BF),
                   preferred_element_type=jnp.float32) + lb3


_pmapped = jax.pmap(_fwd, in_axes=(0, 0, None))


def _build_C(edge_index):
    src = np.asarray(edge_index[0], dtype=np.int64)
    dst = np.asarray(edge_index[1], dtype=np.int64)
    g = dst // NPG
    sl = src - g * NPG
    dl = dst - g * NPG
    idx = (g * NPG + dl) * NPG + sl
    C = np.bincount(idx, minlength=B * NPG * NPG).astype(np.float32)
    C = C.reshape(B, NPG, NPG)
    C[:, _DIAG, _DIAG] += 1.0   # self loops on every node
    return C


def kernel(**inputs):
    x = np.asarray(inputs['x'], np.float32).reshape(B, NPG)
    C = _build_C(inputs['edge_index'])
    pnames = []
    for li in range(1, 5):
        pnames += [f'W{li}', f'as{li}', f'ad{li}', f'b{li}']
    pnames += ['lw1', 'lb1', 'lw2', 'lb2', 'lw3', 'lb3']
    params = tuple(jnp.asarray(np.asarray(inputs[k], np.float32))
                   for k in pnames)
    xs = x.reshape(NC, GPC, NPG)
    Cs = C.reshape(NC, GPC, NPG, NPG)
    out = _pmapped(xs, Cs, params)
    return np.asarray(out).reshape(B, 9).astype(np.float32)
